# revision 10
# baseline (speedup 1.0000x reference)
"""DRConv (dynamic region-aware conv) Trainium2 kernel.

Math (out = sum_t Z_t * U_t + bias, per batch b):
  Z_t = conv3x3(x, template_t)                      # [O, H, W] on device
  U[t,p] = sum_g x_se[g,t] * softmax(Alpha)[g,p]    # per-pixel blend
  x_se = 0.25*sigmoid(routing_w @ mean_hw(x) + rb)  # host (0.002% of flops)
which equals the reference
  out = einsum('boghw,bghw->bohw', einsum('bokg,bkhw->boghw', w, patches),
               softmax(Alpha)) + bias
because w = blend(x_se, templates) commutes through the conv: the blend
weights x_se[g,t] and the softmax probs both act per (g, pixel), so the
G-sum and T-sum exchange with the K-contraction.  Routing (GAP+fc+sigmoid),
softmax(Alpha) (or the one-hot mask when use_alpha=0), and the bias add are
tiny O(n) pre/post-processing done on the host; the device does the 62
GFLOP of convolution and the per-pixel mix.

Sharding: data-parallel over batch B=8, one batch element per NeuronCore.
Templates replicated. No collectives.

Device layout (per core):
  pixels live in a 58x57 plane: one pad row top/bottom, ONE pad column
  (a right-pad column doubles as the left neighbor of the next row's
  x=0 pixel, so 57-wide rows give correct 3x3 zero padding);
  pf = (y+1)*57 + x for image pixel (y, x).  The padded plane (plus a
  64-wide zero guard in front) is built on the HOST and shipped as two
  overlapping bf16 bands so no on-device memset/copy is needed.
  conv = 9 shifted matmuls accumulating in PSUM:
    Z[px, (t,o)] += x[c, guard+base+px+delta(i,j)].T @ tmpl[c, (t,o)]
  pixel tiles are the stationary operand (128 px per matmul), so the
  per-pixel routing mix is 8 scalar_tensor_tensor ops on Vector; the
  output stays in [px, O] layout and the host transposes it back.
  Input DMA (3.3 MB) runs at the ~330 GB/s per-core wall, so transfers
  are strictly priority-ordered (xa, templates, xb) and the first three
  pixel tiles are emitted template-chunk-major so the PE always has
  ready work while later chunks stream in.
"""

import ml_dtypes
import numpy as np

import concourse.bass as bass
import concourse.mybir as mybir
from concourse import bacc
from concourse.tile import TileContext
from concourse.bass_utils import run_bass_kernel_spmd

# problem constants
C = 128          # in channels
O = 128          # out channels
H = W = 56
G = 8            # groups
T = 8            # num weight templates
WP = 57          # padded row width (one shared pad column)
GUARD = 64       # front zero guard for negative conv shifts
PT0 = WP         # first pixel-tile starts at padded row 1
NT = 25          # 25 tiles of 128 px cover pf [57, 3257) > last valid 3247
PLANE = NT * 128  # 3200 output pixels kept on device
HPW = GUARD + 3392  # host plane width (pf -64 .. 3328)
XBW = 1856       # width of band B (bands overlap to cover straddling reads)
XA1W = 960       # band A1: pf [-64, 896), tiles 0-5
XA2W = 1028      # band A2: pf [764, 1792), tiles 6-12
XA2O = 764       # pf origin of band A2
XB1 = 1536       # pf origin of band B
KSPLIT = 13      # tiles k >= KSPLIT read band B
KSPLIT1 = 6      # tiles k < KSPLIT1 read band A1, else A2
WARM = 6        # PE warm-up matmuls (p-state ramp during input DMA)
OCHUNK = [0, 7, 13, 19, 24, 25]  # output store chunk boundaries (tiles)
NCORES = 8

_cache = {}

DELTA = [(i - 1) * WP + (j - 1) for i in range(3) for j in range(3)]


def _build():
    f32 = mybir.dt.float32
    bf16 = mybir.dt.bfloat16

    nc = bacc.Bacc("TRN2", target_bir_lowering=False, debug=False,
                   num_devices=NCORES)

    xa_d = nc.dram_tensor("xa", [C, XA1W], bf16, kind="ExternalInput")
    xc_d = nc.dram_tensor("xc", [C, XA2W], bf16, kind="ExternalInput")
    xb_d = nc.dram_tensor("xb", [C, XBW], bf16, kind="ExternalInput")
    t0a_d = nc.dram_tensor("t0a", [C, T * O], bf16, kind="ExternalInput")
    t0b_d = nc.dram_tensor("t0b", [C, 2 * T * O], bf16,
                           kind="ExternalInput")
    t_d = [nc.dram_tensor(f"t{c}", [C, 3 * T * O], bf16,
                          kind="ExternalInput") for c in range(1, 3)]
    us_d = nc.dram_tensor("us", [128, NT * T], f32, kind="ExternalInput")
    out_d = nc.dram_tensor("out", [128, PLANE], f32, kind="ExternalOutput")

    with TileContext(nc) as tc:
        with (
            tc.tile_pool(name="big", bufs=1) as big,
            tc.tile_pool(name="consts", bufs=1) as consts,
            tc.tile_pool(name="acc", bufs=2) as accp,
            tc.tile_pool(name="zps", bufs=4, space="PSUM") as zps,
        ):
            # PE warm-up source (content irrelevant)
            warm = big.tile([C, 512], bf16)
            nc.vector.memset(warm[:], 0.0)

            # ---- input DMAs: bulk priority-ordered on sync, small on
            # scalar/gpsimd so they steal almost no bandwidth ----
            xba = big.tile([C, XA1W], bf16)
            xbc = big.tile([C, XA2W], bf16)
            xbb = big.tile([C, XBW], bf16)
            tb0a = big.tile([C, T * O], bf16, name="tb0a")
            tb0b = big.tile([C, 2 * T * O], bf16, name="tb0b")
            tbf = [big.tile([C, 3 * T * O], bf16, name=f"tb{c}")
                   for c in range(1, 3)]
            nc.sync.dma_start(out=xba[:], in_=xa_d[:])
            nc.sync.dma_start(out=tb0a[:], in_=t0a_d[:])
            nc.sync.dma_start(out=tb0b[:], in_=t0b_d[:])
            nc.sync.dma_start(out=tbf[0][:], in_=t_d[0][:])
            nc.sync.dma_start(out=xbc[:], in_=xc_d[:])
            nc.sync.dma_start(out=tbf[1][:], in_=t_d[1][:])
            nc.sync.dma_start(out=xbb[:], in_=xb_d[:])
            usbs = big.tile([128, NT * T], f32)
            nc.gpsimd.dma_start(out=usbs[:], in_=us_d[:])

            # ---- PE warm-up (ramps HAM p-state during the DMAs) ----
            wps = zps.tile([128, 512], f32, tag="zp0", name="warmps")
            for _ in range(WARM):
                nc.tensor.matmul(wps[:], lhsT=warm[:, 0:128], rhs=warm[:])

            # ---- output plane [px, O], transposed on host ----
            plane = big.tile([128, PLANE], f32)

            zp = {}
            accs = {}

            def convj(k, ij, hs=(0, 1)):
                base = PT0 + 128 * k
                lo = base + DELTA[ij]
                if k < KSPLIT1:
                    xsl = xba[:, GUARD + lo:GUARD + lo + 128]
                elif k < KSPLIT:
                    xsl = xbc[:, lo - XA2O:lo - XA2O + 128]
                else:
                    xsl = xbb[:, lo - XB1:lo - XB1 + 128]
                if ij == 0:
                    rt, j = tb0a, 0
                elif ij < 3:
                    rt, j = tb0b, ij - 1
                else:
                    rt, j = tbf[ij // 3 - 1], ij % 3
                for h in hs:
                    nc.tensor.matmul(
                        zp[k][h][:],
                        lhsT=xsl,
                        rhs=rt[:, j * 1024 + h * 512:
                               j * 1024 + (h + 1) * 512],
                        start=(ij == 0), stop=(ij == 8))

            def conv6(k, c, hs=(0, 1)):
                for j in range(3):
                    convj(k, 3 * c + j, hs)

            def alloc_zp(k):
                zp[k] = [zps.tile([128, 512], f32, tag=f"zp{h}",
                                  name=f"zp{h}_{k}") for h in range(2)]

            def mix(k, ts):
                if ts[0] == 0:
                    accs[k] = accp.tile([128, O], f32, tag="acc",
                                        name=f"acc{k}")
                acc = accs[k]
                for t in ts:
                    h, tq = divmod(t, 4)
                    src = zp[k][h][:, tq * 128:(tq + 1) * 128]
                    if t == 0:
                        nc.vector.tensor_scalar_mul(
                            acc[:], src, usbs[:, k * T:k * T + 1])
                    else:
                        nc.vector.scalar_tensor_tensor(
                            out=plane[:, k * 128:(k + 1) * 128]
                            if t == T - 1 else acc[:],
                            in0=src,
                            scalar=usbs[:, k * T + t:k * T + t + 1],
                            in1=acc[:],
                            op0=mybir.AluOpType.mult,
                            op1=mybir.AluOpType.add)
                if ts[-1] == T - 1:
                    del zp[k]
                    del accs[k]

            # chunk-major head: tiles 0-2 fill the PE with ready work
            # while later template chunks stream in
            NH = 4
            for k in range(NH):
                alloc_zp(k)
            for kk in range(NH):
                convj(kk, 0)
            for kk in range(NH):
                convj(kk, 1)
                convj(kk, 2)
            for kk in range(NH):
                conv6(kk, 1)
            for kk in range(NH):
                conv6(kk, 2)
                mix(kk, range(T))

            nchunk = 1
            for k in range(NH, NT):
                alloc_zp(k)
                if k < NT - 1:
                    for c in range(3):
                        conv6(k, c)
                    mix(k, range(T))
                else:
                    # last tile: finish PSUM half 0 first so the mix
                    # overlaps the remaining nine matmuls
                    for c in range(3):
                        conv6(k, c, hs=(0,))
                    mix(k, range(4))
                    for c in range(3):
                        conv6(k, c, hs=(1,))
                    mix(k, range(4, T))
                if k + 1 == OCHUNK[nchunk]:
                    lo, hi = OCHUNK[nchunk - 1] * 128, OCHUNK[nchunk] * 128
                    eng = nc.scalar if nchunk == len(OCHUNK) - 1 else nc.sync
                    eng.dma_start(out=out_d[:, lo:hi], in_=plane[:, lo:hi])
                    nchunk += 1

    nc.compile()
    return nc


def _get():
    if "nc" not in _cache:
        _cache["nc"] = _build()
    return _cache["nc"]


def _in_maps(inp):
    ua = int(np.asarray(inp["use_alpha"]))
    x = np.asarray(inp["inputs"], dtype=np.float32)
    # host-padded image plane: pf = (y+1)*57 + x, 64-wide front guard
    hp = np.zeros((NCORES, C, HPW), dtype=ml_dtypes.bfloat16)
    hp[:, :, GUARD + WP:GUARD + WP + H * WP].reshape(
        NCORES, C, H, WP)[:, :, :, 0:W] = x.reshape(NCORES, C, H, W)
    xa = hp[:, :, 0:XA1W]
    xc = hp[:, :, GUARD + XA2O:GUARD + XA2O + XA2W]
    xb = hp[:, :, GUARD + XB1:GUARD + XB1 + XBW]

    # routing: GAP -> fc -> scaled sigmoid (tiny, done on host)
    x_se = x.reshape(NCORES, C, -1).mean(-1) @ np.asarray(
        inp["routing_w"], dtype=np.float32).T + np.asarray(
        inp["routing_b"], dtype=np.float32)
    x_se = 0.25 / (1.0 + np.exp(-x_se))         # (2/T)*sigmoid, T folded

    # routing probabilities (softmax or one-hot)
    if ua:
        a = np.asarray(inp["Alpha"], dtype=np.float32)
        e = np.exp(a - a.max(axis=1, keepdims=True))
        probs = e / e.sum(axis=1, keepdims=True)
    else:
        m = np.asarray(inp["mask"])
        probs = (m[:, None, :, :] == np.arange(G)[None, :, None, None])
        probs = probs.astype(np.float32)
    pp = np.zeros((NCORES, G, 3328), dtype=np.float32)
    pp[:, :, WP:WP + H * WP].reshape(
        NCORES, G, H, WP)[:, :, :, 0:W] = probs
    # U[b, p, k*T+t] = sum_g xse[b,g,t] probs[b,g,pf=57+128k+p]
    us = np.einsum('bgkp,bgt->bpkt',
                   pp[:, :, PT0:PT0 + PLANE].reshape(NCORES, G, NT, 128),
                   x_se.reshape(NCORES, G, T).astype(np.float32),
                   ).reshape(NCORES, 128, NT * T)

    # [O*C*3*3, T] -> 3 chunks of [C, 3*(t*O+o)], ij-major
    t9 = np.asarray(inp["weight_templates"], dtype=np.float32).reshape(
        O, C, 3, 3, T).transpose(2, 3, 1, 4, 0).reshape(9, C, T * O)
    tch = [np.ascontiguousarray(
        t9[3 * c:3 * c + 3].transpose(1, 0, 2).reshape(C, 3 * T * O)
    ).astype(ml_dtypes.bfloat16) for c in range(3)]
    t0a = np.ascontiguousarray(tch[0][:, 0:T * O])
    t0b = np.ascontiguousarray(tch[0][:, T * O:])

    in_maps = []
    for b in range(NCORES):
        in_maps.append({
            "xa": np.ascontiguousarray(xa[b]),
            "xc": np.ascontiguousarray(xc[b]),
            "xb": np.ascontiguousarray(xb[b]),
            "t0a": t0a, "t0b": t0b, "t1": tch[1], "t2": tch[2],
            "us": np.ascontiguousarray(us[b]),
        })
    return in_maps


def kernel(inputs, mask, Alpha, weight_templates, routing_w, routing_b, bias,
           use_alpha):
    nc = _get()
    in_maps = _in_maps(dict(inputs=inputs, mask=mask, Alpha=Alpha,
                            weight_templates=weight_templates,
                            routing_w=routing_w, routing_b=routing_b,
                            bias=bias, use_alpha=use_alpha))
    res = run_bass_kernel_spmd(nc, in_maps, list(range(NCORES)))
    arr = np.stack([res.results[b]["out"] for b in range(NCORES)], axis=0)
    # [b, px_in_tile, (k, o)] -> [b, o, pf-57] -> [b, O, H, W]
    out = arr.reshape(NCORES, 128, NT, O).transpose(0, 3, 2, 1).reshape(
        NCORES, O, PLANE)[:, :, 0:H * WP].reshape(NCORES, O, H, WP)[
        :, :, :, 0:W]
    out = out.astype(np.float32) + np.asarray(
        bias, dtype=np.float32).reshape(1, O, 1, 1)
    return np.ascontiguousarray(out)


# revision 11
# speedup vs baseline: 1.0212x; 1.0212x over previous
"""DRConv (dynamic region-aware conv) Trainium2 kernel.

Math (out = sum_t Z_t * U_t + bias, per batch b):
  Z_t = conv3x3(x, template_t)                      # [O, H, W] on device
  U[t,p] = sum_g x_se[g,t] * softmax(Alpha)[g,p]    # per-pixel blend
  x_se = 0.25*sigmoid(routing_w @ mean_hw(x) + rb)  # host (0.002% of flops)
which equals the reference
  out = einsum('boghw,bghw->bohw', einsum('bokg,bkhw->boghw', w, patches),
               softmax(Alpha)) + bias
because w = blend(x_se, templates) commutes through the conv: the blend
weights x_se[g,t] and the softmax probs both act per (g, pixel), so the
G-sum and T-sum exchange with the K-contraction.  Routing (GAP+fc+sigmoid),
softmax(Alpha) (or the one-hot mask when use_alpha=0), and the bias add are
tiny O(n) pre/post-processing done on the host; the device does the 62
GFLOP of convolution and the per-pixel mix.

Sharding: data-parallel over batch B=8, one batch element per NeuronCore.
Templates replicated. No collectives.

Device layout (per core):
  pixels live in a 58x57 plane: one pad row top/bottom, ONE pad column
  (a right-pad column doubles as the left neighbor of the next row's
  x=0 pixel, so 57-wide rows give correct 3x3 zero padding);
  pf = (y+1)*57 + x for image pixel (y, x).  The padded plane (plus a
  64-wide zero guard in front) is built on the HOST and shipped as two
  overlapping bf16 bands so no on-device memset/copy is needed.
  conv = 9 shifted matmuls accumulating in PSUM:
    Z[px, (t,o)] += x[c, guard+base+px+delta(i,j)].T @ tmpl[c, (t,o)]
  pixel tiles are the stationary operand (128 px per matmul), so the
  per-pixel routing mix is 8 scalar_tensor_tensor ops on Vector; the
  output stays in [px, O] layout and the host transposes it back.
  Input DMA (3.3 MB) runs at the ~330 GB/s per-core wall, so transfers
  are strictly priority-ordered (xa, templates, xb) and the first three
  pixel tiles are emitted template-chunk-major so the PE always has
  ready work while later chunks stream in.
"""

import ml_dtypes
import numpy as np

import concourse.bass as bass
import concourse.mybir as mybir
from concourse import bacc
from concourse.tile import TileContext
from concourse.bass_utils import run_bass_kernel_spmd

# problem constants
C = 128          # in channels
O = 128          # out channels
H = W = 56
G = 8            # groups
T = 8            # num weight templates
WP = 57          # padded row width (one shared pad column)
GUARD = 64       # front zero guard for negative conv shifts
PT0 = WP         # first pixel-tile starts at padded row 1
NT = 25          # 25 tiles of 128 px cover pf [57, 3257) > last valid 3247
PLANE = NT * 128  # 3200 output pixels kept on device
HPW = GUARD + 3392  # host plane width (pf -64 .. 3328)
XBW = 1856       # width of band B (bands overlap to cover straddling reads)
XA1W = 960       # band A1: pf [-64, 896), tiles 0-5
XA2W = 1028      # band A2: pf [764, 1792), tiles 6-12
XA2O = 764       # pf origin of band A2
XB1 = 1536       # pf origin of band B
KSPLIT = 13      # tiles k >= KSPLIT read band B
KSPLIT1 = 6      # tiles k < KSPLIT1 read band A1, else A2
WARM = 9        # PE warm-up matmuls (p-state ramp during input DMA)
OCHUNK = [0, 7, 13, 19, 24, 25]  # output store chunk boundaries (tiles)
NCORES = 8

_cache = {}

DELTA = [(i - 1) * WP + (j - 1) for i in range(3) for j in range(3)]


def _build():
    f32 = mybir.dt.float32
    bf16 = mybir.dt.bfloat16

    nc = bacc.Bacc("TRN2", target_bir_lowering=False, debug=False,
                   num_devices=NCORES)

    xa_d = nc.dram_tensor("xa", [C, XA1W], bf16, kind="ExternalInput")
    xc_d = nc.dram_tensor("xc", [C, XA2W], bf16, kind="ExternalInput")
    xb_d = nc.dram_tensor("xb", [C, XBW], bf16, kind="ExternalInput")
    t0a_d = nc.dram_tensor("t0a", [C, T * O], bf16, kind="ExternalInput")
    t0b_d = nc.dram_tensor("t0b", [C, 2 * T * O], bf16,
                           kind="ExternalInput")
    t_d = [nc.dram_tensor(f"t{c}", [C, 3 * T * O], bf16,
                          kind="ExternalInput") for c in range(1, 3)]
    us_d = nc.dram_tensor("us", [128, NT * T], f32, kind="ExternalInput")
    out_d = nc.dram_tensor("out", [128, PLANE], f32, kind="ExternalOutput")

    with TileContext(nc) as tc:
        with (
            tc.tile_pool(name="big", bufs=1) as big,
            tc.tile_pool(name="consts", bufs=1) as consts,
            tc.tile_pool(name="acc", bufs=2) as accp,
            tc.tile_pool(name="zps", bufs=4, space="PSUM") as zps,
        ):
            # PE warm-up source (content irrelevant)
            warm = big.tile([C, 512], bf16)
            nc.vector.memset(warm[:], 0.0)

            # ---- input DMAs: bulk priority-ordered on sync, small on
            # scalar/gpsimd so they steal almost no bandwidth ----
            xba = big.tile([C, XA1W], bf16)
            xbc = big.tile([C, XA2W], bf16)
            xbb = big.tile([C, XBW], bf16)
            tb0a = big.tile([C, T * O], bf16, name="tb0a")
            tb0b = big.tile([C, 2 * T * O], bf16, name="tb0b")
            tbf = [big.tile([C, 3 * T * O], bf16, name=f"tb{c}")
                   for c in range(1, 3)]
            nc.sync.dma_start(out=xba[:], in_=xa_d[:])
            nc.sync.dma_start(out=tb0a[:], in_=t0a_d[:])
            nc.sync.dma_start(out=tb0b[:], in_=t0b_d[:])
            nc.sync.dma_start(out=tbf[0][:], in_=t_d[0][:])
            nc.sync.dma_start(out=xbc[:], in_=xc_d[:])
            nc.sync.dma_start(out=tbf[1][:], in_=t_d[1][:])
            nc.sync.dma_start(out=xbb[:], in_=xb_d[:])
            usbs = big.tile([128, NT * T], f32)
            nc.gpsimd.dma_start(out=usbs[:], in_=us_d[:])

            # ---- PE warm-up (ramps HAM p-state during the DMAs) ----
            wps = zps.tile([128, 512], f32, tag="zp0", name="warmps")
            for _ in range(WARM):
                nc.tensor.matmul(wps[:], lhsT=warm[:, 0:128], rhs=warm[:])

            # ---- output plane [px, O], transposed on host ----
            plane = big.tile([128, PLANE], f32)

            zp = {}
            accs = {}

            def convj(k, ij, hs=(0, 1)):
                base = PT0 + 128 * k
                lo = base + DELTA[ij]
                if k < KSPLIT1:
                    xsl = xba[:, GUARD + lo:GUARD + lo + 128]
                elif k < KSPLIT:
                    xsl = xbc[:, lo - XA2O:lo - XA2O + 128]
                else:
                    xsl = xbb[:, lo - XB1:lo - XB1 + 128]
                if ij == 0:
                    rt, j = tb0a, 0
                elif ij < 3:
                    rt, j = tb0b, ij - 1
                else:
                    rt, j = tbf[ij // 3 - 1], ij % 3
                for h in hs:
                    nc.tensor.matmul(
                        zp[k][h][:],
                        lhsT=xsl,
                        rhs=rt[:, j * 1024 + h * 512:
                               j * 1024 + (h + 1) * 512],
                        start=(ij == 0), stop=(ij == 8))

            def conv6(k, c, hs=(0, 1)):
                for j in range(3):
                    convj(k, 3 * c + j, hs)

            def alloc_zp(k):
                zp[k] = [zps.tile([128, 512], f32, tag=f"zp{h}",
                                  name=f"zp{h}_{k}") for h in range(2)]

            def mix(k, ts):
                if ts[0] == 0:
                    accs[k] = accp.tile([128, O], f32, tag="acc",
                                        name=f"acc{k}")
                acc = accs[k]
                for t in ts:
                    h, tq = divmod(t, 4)
                    src = zp[k][h][:, tq * 128:(tq + 1) * 128]
                    if t == 0:
                        nc.vector.tensor_scalar_mul(
                            acc[:], src, usbs[:, k * T:k * T + 1])
                    else:
                        nc.vector.scalar_tensor_tensor(
                            out=plane[:, k * 128:(k + 1) * 128]
                            if t == T - 1 else acc[:],
                            in0=src,
                            scalar=usbs[:, k * T + t:k * T + t + 1],
                            in1=acc[:],
                            op0=mybir.AluOpType.mult,
                            op1=mybir.AluOpType.add)
                if ts[-1] == T - 1:
                    del zp[k]
                    del accs[k]

            # chunk-major head: tiles 0-2 fill the PE with ready work
            # while later template chunks stream in
            NH = 4
            for k in range(NH):
                alloc_zp(k)
            for kk in range(NH):
                convj(kk, 0)
            for kk in range(NH):
                convj(kk, 1)
                convj(kk, 2)
            for kk in range(NH):
                conv6(kk, 1)
            for kk in range(NH):
                conv6(kk, 2)
                mix(kk, range(T))

            nchunk = 1
            for k in range(NH, NT):
                alloc_zp(k)
                if k < NT - 1:
                    for c in range(3):
                        conv6(k, c)
                    mix(k, range(T))
                else:
                    # last tile: finish PSUM half 0 first so the mix
                    # overlaps the remaining nine matmuls
                    for c in range(3):
                        conv6(k, c, hs=(0,))
                    mix(k, range(4))
                    for c in range(3):
                        conv6(k, c, hs=(1,))
                    mix(k, range(4, T))
                if k + 1 == OCHUNK[nchunk]:
                    lo, hi = OCHUNK[nchunk - 1] * 128, OCHUNK[nchunk] * 128
                    eng = nc.scalar if nchunk == len(OCHUNK) - 1 else nc.sync
                    eng.dma_start(out=out_d[:, lo:hi], in_=plane[:, lo:hi])
                    nchunk += 1

    nc.compile()
    return nc


def _get():
    if "nc" not in _cache:
        _cache["nc"] = _build()
    return _cache["nc"]


def _in_maps(inp):
    ua = int(np.asarray(inp["use_alpha"]))
    x = np.asarray(inp["inputs"], dtype=np.float32)
    # host-padded image plane: pf = (y+1)*57 + x, 64-wide front guard
    hp = np.zeros((NCORES, C, HPW), dtype=ml_dtypes.bfloat16)
    hp[:, :, GUARD + WP:GUARD + WP + H * WP].reshape(
        NCORES, C, H, WP)[:, :, :, 0:W] = x.reshape(NCORES, C, H, W)
    xa = hp[:, :, 0:XA1W]
    xc = hp[:, :, GUARD + XA2O:GUARD + XA2O + XA2W]
    xb = hp[:, :, GUARD + XB1:GUARD + XB1 + XBW]

    # routing: GAP -> fc -> scaled sigmoid (tiny, done on host)
    x_se = x.reshape(NCORES, C, -1).mean(-1) @ np.asarray(
        inp["routing_w"], dtype=np.float32).T + np.asarray(
        inp["routing_b"], dtype=np.float32)
    x_se = 0.25 / (1.0 + np.exp(-x_se))         # (2/T)*sigmoid, T folded

    # routing probabilities (softmax or one-hot)
    if ua:
        a = np.asarray(inp["Alpha"], dtype=np.float32)
        e = np.exp(a - a.max(axis=1, keepdims=True))
        probs = e / e.sum(axis=1, keepdims=True)
    else:
        m = np.asarray(inp["mask"])
        probs = (m[:, None, :, :] == np.arange(G)[None, :, None, None])
        probs = probs.astype(np.float32)
    pp = np.zeros((NCORES, G, 3328), dtype=np.float32)
    pp[:, :, WP:WP + H * WP].reshape(
        NCORES, G, H, WP)[:, :, :, 0:W] = probs
    # U[b, p, k*T+t] = sum_g xse[b,g,t] probs[b,g,pf=57+128k+p]
    us = np.einsum('bgkp,bgt->bpkt',
                   pp[:, :, PT0:PT0 + PLANE].reshape(NCORES, G, NT, 128),
                   x_se.reshape(NCORES, G, T).astype(np.float32),
                   ).reshape(NCORES, 128, NT * T)

    # [O*C*3*3, T] -> 3 chunks of [C, 3*(t*O+o)], ij-major
    t9 = np.asarray(inp["weight_templates"], dtype=np.float32).reshape(
        O, C, 3, 3, T).transpose(2, 3, 1, 4, 0).reshape(9, C, T * O)
    tch = [np.ascontiguousarray(
        t9[3 * c:3 * c + 3].transpose(1, 0, 2).reshape(C, 3 * T * O)
    ).astype(ml_dtypes.bfloat16) for c in range(3)]
    t0a = np.ascontiguousarray(tch[0][:, 0:T * O])
    t0b = np.ascontiguousarray(tch[0][:, T * O:])

    in_maps = []
    for b in range(NCORES):
        in_maps.append({
            "xa": np.ascontiguousarray(xa[b]),
            "xc": np.ascontiguousarray(xc[b]),
            "xb": np.ascontiguousarray(xb[b]),
            "t0a": t0a, "t0b": t0b, "t1": tch[1], "t2": tch[2],
            "us": np.ascontiguousarray(us[b]),
        })
    return in_maps


def kernel(inputs, mask, Alpha, weight_templates, routing_w, routing_b, bias,
           use_alpha):
    nc = _get()
    in_maps = _in_maps(dict(inputs=inputs, mask=mask, Alpha=Alpha,
                            weight_templates=weight_templates,
                            routing_w=routing_w, routing_b=routing_b,
                            bias=bias, use_alpha=use_alpha))
    res = run_bass_kernel_spmd(nc, in_maps, list(range(NCORES)))
    arr = np.stack([res.results[b]["out"] for b in range(NCORES)], axis=0)
    # [b, px_in_tile, (k, o)] -> [b, o, pf-57] -> [b, O, H, W]
    out = arr.reshape(NCORES, 128, NT, O).transpose(0, 3, 2, 1).reshape(
        NCORES, O, PLANE)[:, :, 0:H * WP].reshape(NCORES, O, H, WP)[
        :, :, :, 0:W]
    out = out.astype(np.float32) + np.asarray(
        bias, dtype=np.float32).reshape(1, O, 1, 1)
    return np.ascontiguousarray(out)
